# revision 13
# baseline (speedup 1.0000x reference)
"""Supervised contrastive loss on 8 Trainium2 NeuronCores.

Strategy (data-parallel over embedding rows, per the sharding hint), with a
label-sorted layout so the masked work collapses to narrow windows:

  - The host sorts rows AND columns by label (the loss is permutation
    invariant). Each core owns 512 sorted rows; each 128-row m-tile's
    same-label partners then live in ONE contiguous column window of at
    most 512 columns (multinomial counts make wider windows essentially
    impossible; asserted on the host).
  - Dense path: bf16 PE matmuls compute the [128, B] similarity slab in
    PSUM chunks; one ACT Exp pass per chunk (scale=1/T) with the fused
    per-row accumulate yields sum_j exp(s_ij). The Exp output is a dead
    store - only the accumulator is consumed.
  - Window path: 4 small matmuls recompute the window's sims (bit-identical
    inputs), then exp / is_equal mask / masked-multiply / row-reduce /
    log1p run on [128, 512] tiles only (~1/8 of the columns):
      sum_same = sum_win mask * exp;  denom = sum_all - sum_same
      slog = sum_win ln(1 + me * (1/denom))   [diagonal included]
  - Per-row loss: cnt_i*ln(denom_i) + slog_i - ln(denom_i + e^{s_ii})
                  - sum_{j same} s_ij + s_ii
    where cnt, s_ii, e^{s_ii} and sum_{j same} s_ij (via class-sum matrix
    G) are exact O(B*D) host precomputes.
  - Each core writes its 512 per-row contributions; the host sums 4096
    values and divides by num_pos (exact, from label counts).
"""

import ml_dtypes
import numpy as np

import concourse.bass as bass
import concourse.bacc as bacc
import concourse.mybir as mybir
import concourse.tile as tile
from concourse.bass_utils import run_bass_kernel_spmd

B = 4096          # total rows
D = 512           # embedding dim
NCORES = 8
BL = B // NCORES  # rows per core
NK = D // 128     # contraction k-tiles
NMT = BL // 128   # output m-tiles per core
CH = 1024         # dense column chunk (2 PSUM banks)
NCH = B // CH     # dense chunks per m-tile row
WIN = 512         # same-label column window per m-tile
TINV = 10.0       # 1 / temperature
F32 = mybir.dt.float32
BF16 = mybir.dt.bfloat16

_CACHE = {}


def _build_nc():
    nc = bacc.Bacc()
    et = nc.dram_tensor("et", [D, B], BF16, kind="ExternalInput")
    elt = nc.dram_tensor("elt", [D, BL], BF16, kind="ExternalInput")
    etwin = nc.dram_tensor("etwin", [NMT, D, WIN], BF16, kind="ExternalInput")
    colwin = nc.dram_tensor("colwin", [NMT, 128, WIN], BF16, kind="ExternalInput")
    meta = nc.dram_tensor("meta", [NMT, 128, 5], F32, kind="ExternalInput")
    out = nc.dram_tensor("out", [NMT, 128, 1], F32, kind="ExternalOutput")

    AF = mybir.ActivationFunctionType
    OP = mybir.AluOpType

    with tile.TileContext(nc) as tc:
        with (
            tc.tile_pool(name="const", bufs=1) as cpool,
            tc.tile_pool(name="psum", bufs=3, space=bass.MemorySpace.PSUM) as ppool,
            tc.tile_pool(name="psumw", bufs=2, space=bass.MemorySpace.PSUM) as pwpool,
            tc.tile_pool(name="chunks", bufs=3) as chpool,
            tc.tile_pool(name="winp", bufs=2) as wpool,
            tc.tile_pool(name="small", bufs=2) as smpool,
        ):
            ets = [cpool.tile([128, B], BF16, tag=f"ets{k}", name=f"ets{k}")
                   for k in range(NK)]
            eltt = [cpool.tile([128, BL], BF16, tag=f"elt{k}", name=f"elt{k}")
                    for k in range(NK)]
            etw_sb = [[cpool.tile([128, WIN], BF16, tag=f"etw{m}_{k}",
                                  name=f"etw{m}_{k}") for k in range(NK)]
                      for m in range(NMT)]
            colw_sb = [cpool.tile([128, WIN], BF16, tag=f"colw{m}", name=f"colw{m}")
                       for m in range(NMT)]
            meta_sb = [cpool.tile([128, 5], F32, tag=f"meta{m}", name=f"meta{m}")
                       for m in range(NMT)]

            # Loads. ets chunk-major on the sync queue so the first matmuls'
            # columns land first; side tensors on the gpsimd queue.
            for k in range(NK):
                nc.gpsimd.dma_start(eltt[k][:], elt[k * 128:(k + 1) * 128, :])
            for m in range(NMT):
                for k in range(NK):
                    nc.gpsimd.dma_start(
                        etw_sb[m][k][:], etwin[m, k * 128:(k + 1) * 128, :])
                nc.gpsimd.dma_start(colw_sb[m][:], colwin[m])
                nc.gpsimd.dma_start(meta_sb[m][:], meta[m])
            for c in range(NCH):
                for k in range(NK):
                    nc.sync.dma_start(
                        ets[k][:, c * CH:(c + 1) * CH],
                        et[k * 128:(k + 1) * 128, c * CH:(c + 1) * CH],
                    )

            sexps, mews, denoms, invs = [], [], [], []
            # ---- Phase A (Exp table set): dense accums + window pipeline --
            for mt in range(NMT):
                rowlab = meta_sb[mt][:, 0:1]
                aparts = smpool.tile([128, NCH], F32, tag="aparts")

                # dense: sum_j exp(s_ij) via fused accumulate, output dead
                for c in range(NCH):
                    psum = ppool.tile([128, CH], F32, tag="psum")
                    for k in range(NK):
                        lhsT = eltt[k][:, mt * 128:(mt + 1) * 128]
                        for h in range(CH // 512):
                            col0 = c * CH + h * 512
                            nc.tensor.matmul(
                                psum[:, h * 512:(h + 1) * 512],
                                lhsT,
                                ets[k][:, col0:col0 + 512],
                                start=(k == 0),
                                stop=(k == NK - 1),
                            )
                    dead = chpool.tile([128, CH], BF16, tag="dead")
                    nc.scalar.activation(
                        dead[:], psum[:], AF.Exp,
                        scale=TINV, accum_out=aparts[:, c:c + 1],
                    )

                # window: recompute the <=512 same-label columns
                psw = pwpool.tile([128, WIN], F32, tag="psw")
                for k in range(NK):
                    nc.tensor.matmul(
                        psw[:],
                        eltt[k][:, mt * 128:(mt + 1) * 128],
                        etw_sb[mt][k][:],
                        start=(k == 0),
                        stop=(k == NK - 1),
                    )
                expw = wpool.tile([128, WIN], F32, tag="expw")
                nc.scalar.activation(expw[:], psw[:], AF.Exp, scale=TINV)
                maskw = wpool.tile([128, WIN], BF16, tag="maskw")
                nc.gpsimd.tensor_scalar(
                    maskw[:], colw_sb[mt][:], rowlab, None, OP.is_equal)
                mew = wpool.tile([128, WIN], F32, tag=f"mew{mt}", name=f"mew{mt}",
                                 bufs=1)
                nc.vector.tensor_tensor(mew[:], expw[:], maskw[:], OP.mult)
                ssame = smpool.tile([128, 1], F32, tag="ssame")
                nc.vector.tensor_reduce(
                    ssame[:], mew[:], mybir.AxisListType.X, OP.add)

                sall = smpool.tile([128, 1], F32, tag="sall")
                nc.vector.tensor_reduce(
                    sall[:], aparts[:], mybir.AxisListType.X, OP.add)
                denom = smpool.tile([128, 1], F32, tag=f"denom{mt}",
                                    name=f"denom{mt}")
                nc.vector.tensor_sub(denom[:], sall[:], ssame[:])
                inv = smpool.tile([128, 1], F32, tag=f"inv{mt}", name=f"inv{mt}")
                nc.vector.reciprocal(inv[:], denom[:])
                mews.append(mew); denoms.append(denom); invs.append(inv)

            # ---- Phase B (Ln table set): all log work batched ----
            lnouts = wpool.tile([128, WIN], BF16, tag="lnout", bufs=1)
            for mt in range(NMT):
                cnt = meta_sb[mt][:, 1:2]
                sii = meta_sb[mt][:, 2:3]
                rds = meta_sb[mt][:, 3:4]
                eii = meta_sb[mt][:, 4:5]
                denom, inv, mew = denoms[mt], invs[mt], mews[mt]

                lnden = smpool.tile([128, 1], F32, tag="lnden")
                nc.scalar.activation(lnden[:], denom[:], AF.Ln)
                lndiag = smpool.tile([128, 1], F32, tag="lndiag")
                nc.scalar.activation(lndiag[:], eii, AF.Ln, bias=denom[:])
                slog = smpool.tile([128, 1], F32, tag="slog")
                nc.scalar.activation(
                    lnouts[:], mew[:], AF.Ln,
                    scale=inv[:], bias=1.0, accum_out=slog[:],
                )

                # rowtot = cnt*lnden + slog - rds - lndiag + sii
                t1 = smpool.tile([128, 1], F32, tag="t1")
                nc.vector.tensor_tensor(t1[:], cnt, lnden[:], OP.mult)
                t2 = smpool.tile([128, 1], F32, tag="t2")
                nc.vector.tensor_add(t2[:], t1[:], slog[:])
                t3 = smpool.tile([128, 1], F32, tag="t3")
                nc.vector.tensor_sub(t3[:], t2[:], rds)
                t4 = smpool.tile([128, 1], F32, tag="t4")
                nc.vector.tensor_sub(t4[:], t3[:], lndiag[:])
                rowtot = smpool.tile([128, 1], F32, tag="rowtot")
                nc.vector.tensor_add(rowtot[:], t4[:], sii)
                nc.sync.dma_start(out[mt], rowtot[:])
    nc.compile()
    return nc


def _make_in_maps(embeddings, labels):
    """Host-side prep: label-sort, transposes, windows, per-row scalars,
    per-core input dicts. Returns (in_maps, num_pos)."""
    emb0 = np.ascontiguousarray(np.asarray(embeddings, dtype=np.float32))
    lab0 = np.asarray(labels).astype(np.int64)
    assert emb0.shape == (B, D) and lab0.shape == (B,)

    perm = np.argsort(lab0, kind="stable")
    emb = emb0[perm]
    lab = lab0[perm]

    ET = np.ascontiguousarray(emb.T)                      # [D, B], sorted cols
    ET16 = ET.astype(ml_dtypes.bfloat16)
    labf = lab.astype(np.float32)
    lab16 = labf.astype(ml_dtypes.bfloat16)

    ncls = int(lab.max()) + 1
    counts = np.bincount(lab, minlength=ncls)
    cum = np.concatenate([[0], np.cumsum(counts)])
    cnt = counts[lab].astype(np.float64)                  # same-label count incl. self
    num_pos = float(cnt.sum() - B)

    emb64 = emb.astype(np.float64)
    G = np.zeros((ncls, D), np.float64)
    np.add.at(G, lab, emb64)
    rds = (emb64 * G[lab]).sum(1) * TINV                  # sum_{j same} sims_ij / T
    sii = (emb64 * emb64).sum(1) * TINV                   # sims_ii / T

    meta_all = np.stack(
        [labf.astype(np.float64), cnt, sii, rds, np.exp(sii)], axis=-1
    ).astype(np.float32)                                  # [B, 5]

    in_maps = []
    for c in range(NCORES):
        sl = slice(c * BL, (c + 1) * BL)
        etwin = np.zeros((NMT, D, WIN), ml_dtypes.bfloat16)
        colwin = np.zeros((NMT, 128, WIN), ml_dtypes.bfloat16)
        colwin[:, :, :] = ml_dtypes.bfloat16(-1.0)        # never matches a label
        for m in range(NMT):
            r0 = c * BL + m * 128
            c0 = int(cum[lab[r0]])
            c1 = int(cum[lab[r0 + 127] + 1])
            w = c1 - c0
            assert w <= WIN, f"window {w} exceeds {WIN}; rebuild with larger WIN"
            etwin[m, :, :w] = ET16[:, c0:c1]
            colwin[m, :, :w] = lab16[c0:c1][None, :]
        in_maps.append({
            "et": ET16,
            "elt": np.ascontiguousarray(ET16[:, sl]),
            "etwin": etwin,
            "colwin": colwin,
            "meta": np.ascontiguousarray(meta_all[sl].reshape(NMT, 128, 5)),
        })
    return in_maps, num_pos


def kernel(embeddings, labels):
    in_maps, num_pos = _make_in_maps(embeddings, labels)
    if "nc" not in _CACHE:
        _CACHE["nc"] = _build_nc()
    nc = _CACHE["nc"]
    res = run_bass_kernel_spmd(nc, in_maps, list(range(NCORES)))
    total = sum(float(r["out"].sum()) for r in res.results)
    return np.asarray(total / max(num_pos, 1.0), dtype=np.float32)


# revision 16
# speedup vs baseline: 1.1820x; 1.1820x over previous
"""Supervised contrastive loss on 8 Trainium2 NeuronCores.

Strategy (data-parallel over embedding rows, per the sharding hint), with a
label-sorted layout so the masked work collapses to narrow windows:

  - The host sorts rows AND columns by label (the loss is permutation
    invariant). Each core owns 512 sorted rows; each 128-row m-tile's
    same-label partners then live in ONE contiguous column window of at
    most 512 columns (multinomial counts make wider windows essentially
    impossible; asserted on the host).
  - Dense path: bf16 PE matmuls compute the [128, B] similarity slab in
    PSUM chunks; one ACT Exp pass per chunk (scale=1/T) with the fused
    per-row accumulate yields sum_j exp(s_ij). The Exp output is a dead
    store - only the accumulator is consumed.
  - Window path: 4 small matmuls recompute the window's sims (bit-identical
    inputs), then exp / is_equal mask / masked-multiply / row-reduce /
    log1p run on [128, 512] tiles only (~1/8 of the columns):
      sum_same = sum_win mask * exp;  denom = sum_all - sum_same
      slog = sum_win ln(1 + me * (1/denom))   [diagonal included]
  - Per-row loss: cnt_i*ln(denom_i) + slog_i - ln(denom_i + e^{s_ii})
                  - sum_{j same} s_ij + s_ii
    where cnt, s_ii, e^{s_ii} and sum_{j same} s_ij (via class-sum matrix
    G) are exact O(B*D) host precomputes.
  - Each core writes its 512 per-row contributions; the host sums 4096
    values and divides by num_pos (exact, from label counts).
"""

import ml_dtypes
import numpy as np

import concourse.bass as bass
import concourse.bacc as bacc
import concourse.mybir as mybir
import concourse.tile as tile
from concourse.bass_utils import run_bass_kernel_spmd

B = 4096          # total rows
D = 512           # embedding dim
NCORES = 8
BL = B // NCORES  # rows per core
NK = D // 128     # contraction k-tiles
NMT = BL // 128   # output m-tiles per core
CH = 1024         # dense column chunk (2 PSUM banks)
NCH = B // CH     # dense chunks per m-tile row
WIN = 512         # same-label column window per m-tile
TINV = 10.0       # 1 / temperature
F32 = mybir.dt.float32
BF16 = mybir.dt.bfloat16

_CACHE = {}


def _build_nc():
    nc = bacc.Bacc()
    et = nc.dram_tensor("et", [D, B], BF16, kind="ExternalInput")
    elt = nc.dram_tensor("elt", [D, BL], BF16, kind="ExternalInput")
    etwin = nc.dram_tensor("etwin", [NMT, 128, NK * WIN], BF16, kind="ExternalInput")
    colwin = nc.dram_tensor("colwin", [NMT, 128, WIN], BF16, kind="ExternalInput")
    meta = nc.dram_tensor("meta", [NMT, 128, 5], F32, kind="ExternalInput")
    out = nc.dram_tensor("out", [NMT, 128, 1], F32, kind="ExternalOutput")

    AF = mybir.ActivationFunctionType
    OP = mybir.AluOpType

    with tile.TileContext(nc) as tc:
        with (
            tc.tile_pool(name="const", bufs=1) as cpool,
            tc.tile_pool(name="psum", bufs=3, space=bass.MemorySpace.PSUM) as ppool,
            tc.tile_pool(name="psumw", bufs=2, space=bass.MemorySpace.PSUM) as pwpool,
            tc.tile_pool(name="chunks", bufs=3) as chpool,
            tc.tile_pool(name="winp", bufs=2) as wpool,
            tc.tile_pool(name="small", bufs=2) as smpool,
        ):
            ets = [cpool.tile([128, B], BF16, tag=f"ets{k}", name=f"ets{k}")
                   for k in range(NK)]
            eltt = [cpool.tile([128, BL], BF16, tag=f"elt{k}", name=f"elt{k}")
                    for k in range(NK)]
            etw_sb = [cpool.tile([128, NK * WIN], BF16, tag=f"etw{m}",
                                 name=f"etw{m}") for m in range(NMT)]
            colw_sb = [cpool.tile([128, WIN], BF16, tag=f"colw{m}", name=f"colw{m}")
                       for m in range(NMT)]
            meta_sb = [cpool.tile([128, 5], F32, tag=f"meta{m}", name=f"meta{m}")
                       for m in range(NMT)]

            # Loads, all on the sync HWDGE queue, interleaved so each
            # m-tile's window data follows the dense columns it needs first.
            for k in range(NK):
                nc.sync.dma_start(eltt[k][:], elt[k * 128:(k + 1) * 128, :])
            for c in range(NCH):
                for k in range(NK):
                    nc.sync.dma_start(
                        ets[k][:, c * CH:(c + 1) * CH],
                        et[k * 128:(k + 1) * 128, c * CH:(c + 1) * CH],
                    )
                m = c
                nc.sync.dma_start(etw_sb[m][:], etwin[m])
                nc.sync.dma_start(colw_sb[m][:], colwin[m])
                nc.sync.dma_start(meta_sb[m][:], meta[m])

            sexps, mews, denoms, invs = [], [], [], []
            # ---- Phase A (Exp table set): dense accums + window pipeline --
            for mt in range(NMT):
                rowlab = meta_sb[mt][:, 0:1]
                aparts = smpool.tile([128, NCH], F32, tag="aparts")

                # dense: sum_j exp(s_ij) via fused accumulate, output dead
                for c in range(NCH):
                    psum = ppool.tile([128, CH], F32, tag="psum")
                    for k in range(NK):
                        lhsT = eltt[k][:, mt * 128:(mt + 1) * 128]
                        for h in range(CH // 512):
                            col0 = c * CH + h * 512
                            nc.tensor.matmul(
                                psum[:, h * 512:(h + 1) * 512],
                                lhsT,
                                ets[k][:, col0:col0 + 512],
                                start=(k == 0),
                                stop=(k == NK - 1),
                            )
                    dead = chpool.tile([128, CH], BF16, tag="dead")
                    nc.scalar.activation(
                        dead[:], psum[:], AF.Exp,
                        scale=TINV, accum_out=aparts[:, c:c + 1],
                    )

                # window: recompute the <=512 same-label columns
                psw = pwpool.tile([128, WIN], F32, tag="psw")
                for k in range(NK):
                    nc.tensor.matmul(
                        psw[:],
                        eltt[k][:, mt * 128:(mt + 1) * 128],
                        etw_sb[mt][:, k * WIN:(k + 1) * WIN],
                        start=(k == 0),
                        stop=(k == NK - 1),
                    )
                expw = wpool.tile([128, WIN], F32, tag="expw")
                last_a_act = nc.scalar.activation(
                    expw[:], psw[:], AF.Exp, scale=TINV)
                maskw = wpool.tile([128, WIN], BF16, tag="maskw")
                nc.vector.tensor_scalar(
                    maskw[:], colw_sb[mt][:], rowlab, None, OP.is_equal)
                mew = wpool.tile([128, WIN], F32, tag=f"mew{mt}", name=f"mew{mt}",
                                 bufs=1)
                nc.vector.tensor_tensor(mew[:], expw[:], maskw[:], OP.mult)
                ssame = smpool.tile([128, 1], F32, tag="ssame")
                nc.vector.tensor_reduce(
                    ssame[:], mew[:], mybir.AxisListType.X, OP.add)

                sall = smpool.tile([128, 1], F32, tag="sall")
                nc.vector.tensor_reduce(
                    sall[:], aparts[:], mybir.AxisListType.X, OP.add)
                denom = smpool.tile([128, 1], F32, tag=f"denom{mt}",
                                    name=f"denom{mt}")
                nc.vector.tensor_sub(denom[:], sall[:], ssame[:])
                inv = smpool.tile([128, 1], F32, tag=f"inv{mt}", name=f"inv{mt}")
                nc.vector.reciprocal(inv[:], denom[:])
                mews.append(mew); denoms.append(denom); invs.append(inv)

            # ---- Phase B (Ln table set): all log work batched ----
            lnouts = wpool.tile([128, WIN], BF16, tag="lnout", bufs=1)
            for mt in range(NMT):
                cnt = meta_sb[mt][:, 1:2]
                sii = meta_sb[mt][:, 2:3]
                rds = meta_sb[mt][:, 3:4]
                eii = meta_sb[mt][:, 4:5]
                denom, inv, mew = denoms[mt], invs[mt], mews[mt]

                lnden = smpool.tile([128, 1], F32, tag="lnden")
                i_ld = nc.scalar.activation(lnden[:], denom[:], AF.Ln)
                tile.add_dep_helper(i_ld.ins, last_a_act.ins, sync=False,
                                    reason="keep Ln set after all Exp work")
                lndiag = smpool.tile([128, 1], F32, tag="lndiag")
                nc.scalar.activation(lndiag[:], eii, AF.Ln, bias=denom[:])
                slog = smpool.tile([128, 1], F32, tag="slog")
                nc.scalar.activation(
                    lnouts[:], mew[:], AF.Ln,
                    scale=inv[:], bias=1.0, accum_out=slog[:],
                )

                # rowtot = cnt*lnden + slog - rds - lndiag + sii
                t1 = smpool.tile([128, 1], F32, tag="t1")
                nc.vector.tensor_tensor(t1[:], cnt, lnden[:], OP.mult)
                t2 = smpool.tile([128, 1], F32, tag="t2")
                nc.vector.tensor_add(t2[:], t1[:], slog[:])
                t3 = smpool.tile([128, 1], F32, tag="t3")
                nc.vector.tensor_sub(t3[:], t2[:], rds)
                t4 = smpool.tile([128, 1], F32, tag="t4")
                nc.vector.tensor_sub(t4[:], t3[:], lndiag[:])
                rowtot = smpool.tile([128, 1], F32, tag="rowtot")
                nc.vector.tensor_add(rowtot[:], t4[:], sii)
                nc.sync.dma_start(out[mt], rowtot[:])
    nc.compile()
    return nc


def _make_in_maps(embeddings, labels):
    """Host-side prep: label-sort, transposes, windows, per-row scalars,
    per-core input dicts. Returns (in_maps, num_pos)."""
    emb0 = np.ascontiguousarray(np.asarray(embeddings, dtype=np.float32))
    lab0 = np.asarray(labels).astype(np.int64)
    assert emb0.shape == (B, D) and lab0.shape == (B,)

    perm = np.argsort(lab0, kind="stable")
    emb = emb0[perm]
    lab = lab0[perm]

    ET = np.ascontiguousarray(emb.T)                      # [D, B], sorted cols
    ET16 = ET.astype(ml_dtypes.bfloat16)
    labf = lab.astype(np.float32)
    lab16 = labf.astype(ml_dtypes.bfloat16)

    ncls = int(lab.max()) + 1
    counts = np.bincount(lab, minlength=ncls)
    cum = np.concatenate([[0], np.cumsum(counts)])
    cnt = counts[lab].astype(np.float64)                  # same-label count incl. self
    num_pos = float(cnt.sum() - B)

    emb64 = emb.astype(np.float64)
    G = np.zeros((ncls, D), np.float64)
    np.add.at(G, lab, emb64)
    rds = (emb64 * G[lab]).sum(1) * TINV                  # sum_{j same} sims_ij / T
    sii = (emb64 * emb64).sum(1) * TINV                   # sims_ii / T

    meta_all = np.stack(
        [labf.astype(np.float64), cnt, sii, rds, np.exp(sii)], axis=-1
    ).astype(np.float32)                                  # [B, 5]

    in_maps = []
    for c in range(NCORES):
        sl = slice(c * BL, (c + 1) * BL)
        etwin = np.zeros((NMT, D, WIN), ml_dtypes.bfloat16)
        colwin = np.zeros((NMT, 128, WIN), ml_dtypes.bfloat16)
        colwin[:, :, :] = ml_dtypes.bfloat16(-1.0)        # never matches a label
        for m in range(NMT):
            r0 = c * BL + m * 128
            c0 = int(cum[lab[r0]])
            c1 = int(cum[lab[r0 + 127] + 1])
            w = c1 - c0
            assert w <= WIN, f"window {w} exceeds {WIN}; rebuild with larger WIN"
            etwin[m, :, :w] = ET16[:, c0:c1]
            colwin[m, :, :w] = lab16[c0:c1][None, :]
        etwin_packed = np.ascontiguousarray(
            etwin.reshape(NMT, NK, 128, WIN).transpose(0, 2, 1, 3)
            .reshape(NMT, 128, NK * WIN))
        in_maps.append({
            "et": ET16,
            "elt": np.ascontiguousarray(ET16[:, sl]),
            "etwin": etwin_packed,
            "colwin": colwin,
            "meta": np.ascontiguousarray(meta_all[sl].reshape(NMT, 128, 5)),
        })
    return in_maps, num_pos


def kernel(embeddings, labels):
    in_maps, num_pos = _make_in_maps(embeddings, labels)
    if "nc" not in _CACHE:
        _CACHE["nc"] = _build_nc()
    nc = _CACHE["nc"]
    res = run_bass_kernel_spmd(nc, in_maps, list(range(NCORES)))
    total = sum(float(r["out"].sum()) for r in res.results)
    return np.asarray(total / max(num_pos, 1.0), dtype=np.float32)


# revision 18
# speedup vs baseline: 1.2014x; 1.0164x over previous
"""Supervised contrastive loss on 8 Trainium2 NeuronCores.

Strategy (data-parallel over embedding rows, per the sharding hint), with a
label-sorted layout so the masked work collapses to narrow windows:

  - The host sorts rows AND columns by label (the loss is permutation
    invariant). Each core owns 512 sorted rows; each 128-row m-tile's
    same-label partners then live in ONE contiguous column window of at
    most 512 columns (multinomial counts make wider windows essentially
    impossible; asserted on the host).
  - Dense path: bf16 PE matmuls compute the [128, B] similarity slab in
    PSUM chunks; one ACT Exp pass per chunk (scale=1/T) with the fused
    per-row accumulate yields sum_j exp(s_ij). The Exp output is a dead
    store - only the accumulator is consumed.
  - Window path: 4 small matmuls recompute the window's sims (bit-identical
    inputs), then exp / is_equal mask / masked-multiply / row-reduce /
    log1p run on [128, 512] tiles only (~1/8 of the columns):
      sum_same = sum_win mask * exp;  denom = sum_all - sum_same
      slog = sum_win ln(1 + me * (1/denom))   [diagonal included]
  - Per-row loss: cnt_i*ln(denom_i) + slog_i - ln(denom_i + e^{s_ii})
                  - sum_{j same} s_ij + s_ii
    where cnt, s_ii, e^{s_ii} and sum_{j same} s_ij (via class-sum matrix
    G) are exact O(B*D) host precomputes.
  - Each core writes its 512 per-row contributions; the host sums 4096
    values and divides by num_pos (exact, from label counts).
"""

import ml_dtypes
import numpy as np

import concourse.bass as bass
import concourse.bacc as bacc
import concourse.mybir as mybir
import concourse.tile as tile
from concourse.bass_utils import run_bass_kernel_spmd

B = 4096          # total rows
D = 512           # embedding dim
NCORES = 8
BL = B // NCORES  # rows per core
NK = D // 128     # contraction k-tiles
NMT = BL // 128   # output m-tiles per core
CH = 1024         # dense column chunk (2 PSUM banks)
NCH = B // CH     # dense chunks per m-tile row
WIN = 512         # same-label column window per m-tile
TINV = 10.0       # 1 / temperature
F32 = mybir.dt.float32
BF16 = mybir.dt.bfloat16

_CACHE = {}


def _build_nc():
    nc = bacc.Bacc()
    et = nc.dram_tensor("et", [D, B], BF16, kind="ExternalInput")
    elt = nc.dram_tensor("elt", [D, BL], BF16, kind="ExternalInput")
    etwin = nc.dram_tensor("etwin", [NMT, 128, NK * WIN], BF16, kind="ExternalInput")
    colwin = nc.dram_tensor("colwin", [NMT, 128, WIN], BF16, kind="ExternalInput")
    meta = nc.dram_tensor("meta", [NMT, 128, 4], F32, kind="ExternalInput")
    out = nc.dram_tensor("out", [NMT, 128, 1], F32, kind="ExternalOutput")

    AF = mybir.ActivationFunctionType
    OP = mybir.AluOpType

    with tile.TileContext(nc) as tc:
        with (
            tc.tile_pool(name="const", bufs=1) as cpool,
            tc.tile_pool(name="psum", bufs=3, space=bass.MemorySpace.PSUM) as ppool,
            tc.tile_pool(name="psumw", bufs=2, space=bass.MemorySpace.PSUM) as pwpool,
            tc.tile_pool(name="chunks", bufs=3) as chpool,
            tc.tile_pool(name="winp", bufs=2) as wpool,
            tc.tile_pool(name="small", bufs=2) as smpool,
        ):
            ets = [cpool.tile([128, B], BF16, tag=f"ets{k}", name=f"ets{k}")
                   for k in range(NK)]
            eltt = [cpool.tile([128, BL], BF16, tag=f"elt{k}", name=f"elt{k}")
                    for k in range(NK)]
            etw_sb = [cpool.tile([128, NK * WIN], BF16, tag=f"etw{m}",
                                 name=f"etw{m}") for m in range(NMT)]
            colw_sb = [cpool.tile([128, WIN], BF16, tag=f"colw{m}", name=f"colw{m}")
                       for m in range(NMT)]
            meta_sb = [cpool.tile([128, 4], F32, tag=f"meta{m}", name=f"meta{m}")
                       for m in range(NMT)]

            # Loads, spread over three queues so issue serialization does
            # not gate the first matmuls: lhsT on gpsimd, dense rhs halves
            # alternating sync/gpsimd, window data on the vector queue.
            for k in range(NK):
                nc.gpsimd.dma_start(eltt[k][:], elt[k * 128:(k + 1) * 128, :])
            HB = B // 2
            for half in range(2):
                for k in range(NK):
                    eng = nc.sync if k % 2 == 0 else nc.gpsimd
                    eng.dma_start(
                        ets[k][:, half * HB:(half + 1) * HB],
                        et[k * 128:(k + 1) * 128, half * HB:(half + 1) * HB],
                    )
            for m in range(NMT):
                nc.scalar.dma_start(meta_sb[m][:], meta[m])
                nc.scalar.dma_start(colw_sb[m][:], colwin[m])
                nc.scalar.dma_start(etw_sb[m][:], etwin[m])

            sexps, mews, denoms, invs = [], [], [], []
            # ---- Phase A (Exp table set): dense accums + window pipeline --
            for mt in range(NMT):
                rowlab = meta_sb[mt][:, 0:1]
                aparts = smpool.tile([128, NCH], F32, tag="aparts")

                # dense: sum_j exp(s_ij) via fused accumulate, output dead
                for c in range(NCH):
                    psum = ppool.tile([128, CH], F32, tag="psum")
                    for k in range(NK):
                        lhsT = eltt[k][:, mt * 128:(mt + 1) * 128]
                        for h in range(CH // 512):
                            col0 = c * CH + h * 512
                            nc.tensor.matmul(
                                psum[:, h * 512:(h + 1) * 512],
                                lhsT,
                                ets[k][:, col0:col0 + 512],
                                start=(k == 0),
                                stop=(k == NK - 1),
                            )
                    dead = chpool.tile([128, CH], BF16, tag="dead")
                    nc.scalar.activation(
                        dead[:], psum[:], AF.Exp,
                        scale=TINV, accum_out=aparts[:, c:c + 1],
                    )

                # window: recompute the <=512 same-label columns
                psw = pwpool.tile([128, WIN], F32, tag="psw")
                for k in range(NK):
                    nc.tensor.matmul(
                        psw[:],
                        eltt[k][:, mt * 128:(mt + 1) * 128],
                        etw_sb[mt][:, k * WIN:(k + 1) * WIN],
                        start=(k == 0),
                        stop=(k == NK - 1),
                    )
                expw = wpool.tile([128, WIN], F32, tag="expw")
                last_a_act = nc.scalar.activation(
                    expw[:], psw[:], AF.Exp, scale=TINV)
                maskw = wpool.tile([128, WIN], BF16, tag="maskw")
                nc.vector.tensor_scalar(
                    maskw[:], colw_sb[mt][:], rowlab, None, OP.is_equal)
                mew = wpool.tile([128, WIN], F32, tag=f"mew{mt}", name=f"mew{mt}",
                                 bufs=1)
                nc.vector.tensor_tensor(mew[:], expw[:], maskw[:], OP.mult)
                ssame = smpool.tile([128, 1], F32, tag="ssame")
                nc.vector.tensor_reduce(
                    ssame[:], mew[:], mybir.AxisListType.X, OP.add)

                sall = smpool.tile([128, 1], F32, tag="sall")
                nc.vector.tensor_reduce(
                    sall[:], aparts[:], mybir.AxisListType.X, OP.add)
                denom = smpool.tile([128, 1], F32, tag=f"denom{mt}",
                                    name=f"denom{mt}")
                nc.vector.tensor_sub(denom[:], sall[:], ssame[:])
                inv = smpool.tile([128, 1], F32, tag=f"inv{mt}", name=f"inv{mt}")
                nc.vector.reciprocal(inv[:], denom[:])
                mews.append(mew); denoms.append(denom); invs.append(inv)

            # ---- Phase B (Ln table set): all log work batched ----
            lnouts = wpool.tile([128, WIN], BF16, tag="lnout", bufs=1)
            for mt in range(NMT):
                cnt = meta_sb[mt][:, 1:2]
                msum = meta_sb[mt][:, 2:3]
                eii = meta_sb[mt][:, 3:4]
                denom, inv, mew = denoms[mt], invs[mt], mews[mt]

                lnden = smpool.tile([128, 1], F32, tag=f"lnden{mt}",
                                    name=f"lnden{mt}")
                i_ld = nc.scalar.activation(lnden[:], denom[:], AF.Ln)
                tile.add_dep_helper(i_ld.ins, last_a_act.ins, sync=False,
                                    reason="keep Ln set after all Exp work")
                lndiag = smpool.tile([128, 1], F32, tag=f"lndiag{mt}",
                                     name=f"lndiag{mt}")
                i_lg = nc.scalar.activation(lndiag[:], eii, AF.Ln, bias=denom[:])
                tile.add_dep_helper(i_lg.ins, last_a_act.ins, sync=False,
                                    reason="keep Ln set after all Exp work")
                slog = smpool.tile([128, 1], F32, tag=f"slog{mt}",
                                   name=f"slog{mt}")
                i_sl = nc.scalar.activation(
                    lnouts[:], mew[:], AF.Ln,
                    scale=inv[:], bias=1.0, accum_out=slog[:],
                )
                tile.add_dep_helper(i_sl.ins, last_a_act.ins, sync=False,
                                    reason="keep Ln set after all Exp work")

                # rowtot = ((cnt*lnden + slog) - lndiag) + (sii - rds)
                ta = smpool.tile([128, 1], F32, tag=f"ta{mt}", name=f"ta{mt}")
                nc.vector.tensor_scalar(
                    ta[:], lnden[:], cnt, slog[:, 0:1], OP.mult, OP.add)
                rowtot = smpool.tile([128, 1], F32, tag=f"rowtot{mt}",
                                     name=f"rowtot{mt}")
                nc.vector.tensor_scalar(
                    rowtot[:], ta[:], lndiag[:, 0:1], msum, OP.subtract, OP.add)
                nc.sync.dma_start(out[mt], rowtot[:])
    nc.compile()
    return nc


def _make_in_maps(embeddings, labels):
    """Host-side prep: label-sort, transposes, windows, per-row scalars,
    per-core input dicts. Returns (in_maps, num_pos)."""
    emb0 = np.ascontiguousarray(np.asarray(embeddings, dtype=np.float32))
    lab0 = np.asarray(labels).astype(np.int64)
    assert emb0.shape == (B, D) and lab0.shape == (B,)

    perm = np.argsort(lab0, kind="stable")
    emb = emb0[perm]
    lab = lab0[perm]

    ET = np.ascontiguousarray(emb.T)                      # [D, B], sorted cols
    ET16 = ET.astype(ml_dtypes.bfloat16)
    labf = lab.astype(np.float32)
    lab16 = labf.astype(ml_dtypes.bfloat16)

    ncls = int(lab.max()) + 1
    counts = np.bincount(lab, minlength=ncls)
    cum = np.concatenate([[0], np.cumsum(counts)])
    cnt = counts[lab].astype(np.float64)                  # same-label count incl. self
    num_pos = float(cnt.sum() - B)

    emb64 = emb.astype(np.float64)
    G = np.zeros((ncls, D), np.float64)
    np.add.at(G, lab, emb64)
    rds = (emb64 * G[lab]).sum(1) * TINV                  # sum_{j same} sims_ij / T
    sii = (emb64 * emb64).sum(1) * TINV                   # sims_ii / T

    meta_all = np.stack(
        [labf.astype(np.float64), cnt, sii - rds, np.exp(sii)], axis=-1
    ).astype(np.float32)                                  # [B, 4]

    in_maps = []
    for c in range(NCORES):
        sl = slice(c * BL, (c + 1) * BL)
        etwin = np.zeros((NMT, D, WIN), ml_dtypes.bfloat16)
        colwin = np.zeros((NMT, 128, WIN), ml_dtypes.bfloat16)
        colwin[:, :, :] = ml_dtypes.bfloat16(-1.0)        # never matches a label
        for m in range(NMT):
            r0 = c * BL + m * 128
            c0 = int(cum[lab[r0]])
            c1 = int(cum[lab[r0 + 127] + 1])
            w = c1 - c0
            assert w <= WIN, f"window {w} exceeds {WIN}; rebuild with larger WIN"
            etwin[m, :, :w] = ET16[:, c0:c1]
            colwin[m, :, :w] = lab16[c0:c1][None, :]
        etwin_packed = np.ascontiguousarray(
            etwin.reshape(NMT, NK, 128, WIN).transpose(0, 2, 1, 3)
            .reshape(NMT, 128, NK * WIN))
        in_maps.append({
            "et": ET16,
            "elt": np.ascontiguousarray(ET16[:, sl]),
            "etwin": etwin_packed,
            "colwin": colwin,
            "meta": np.ascontiguousarray(meta_all[sl].reshape(NMT, 128, 4)),
        })
    return in_maps, num_pos


def kernel(embeddings, labels):
    in_maps, num_pos = _make_in_maps(embeddings, labels)
    if "nc" not in _CACHE:
        _CACHE["nc"] = _build_nc()
    nc = _CACHE["nc"]
    res = run_bass_kernel_spmd(nc, in_maps, list(range(NCORES)))
    total = sum(float(r["out"].sum()) for r in res.results)
    return np.asarray(total / max(num_pos, 1.0), dtype=np.float32)


# revision 19
# speedup vs baseline: 1.3713x; 1.1414x over previous
"""Supervised contrastive loss on 8 Trainium2 NeuronCores.

Strategy (data-parallel over embedding rows, per the sharding hint), with a
label-sorted layout so the masked work collapses to narrow windows:

  - The host sorts rows AND columns by label (the loss is permutation
    invariant). Each core owns 512 sorted rows; each 128-row m-tile's
    same-label partners then live in ONE contiguous column window of at
    most 512 columns (multinomial counts make wider windows essentially
    impossible; asserted on the host).
  - Dense path: bf16 PE matmuls compute the [128, B] similarity slab in
    PSUM chunks; one ACT Exp pass per chunk (scale=1/T) with the fused
    per-row accumulate yields sum_j exp(s_ij). The Exp output is a dead
    store - only the accumulator is consumed.
  - Window path: 4 small matmuls recompute the window's sims (bit-identical
    inputs), then exp / is_equal mask / masked-multiply / row-reduce /
    log1p run on [128, 512] tiles only (~1/8 of the columns):
      sum_same = sum_win mask * exp;  denom = sum_all - sum_same
      slog = sum_win ln(1 + me * (1/denom))   [diagonal included]
  - Per-row loss: cnt_i*ln(denom_i) + slog_i - ln(denom_i + e^{s_ii})
                  - sum_{j same} s_ij + s_ii
    where cnt, s_ii, e^{s_ii} and sum_{j same} s_ij (via class-sum matrix
    G) are exact O(B*D) host precomputes.
  - Each core writes its 512 per-row contributions; the host sums 4096
    values and divides by num_pos (exact, from label counts).
"""

import ml_dtypes
import numpy as np

import concourse.bass as bass
import concourse.bacc as bacc
import concourse.mybir as mybir
import concourse.tile as tile
from concourse.bass_utils import run_bass_kernel_spmd

B = 4096          # total rows
D = 512           # embedding dim
NCORES = 8
BL = B // NCORES  # rows per core
NK = D // 128     # contraction k-tiles
NMT = BL // 128   # output m-tiles per core
CH = 1024         # dense column chunk (2 PSUM banks)
NCH = B // CH     # dense chunks per m-tile row
WIN = 512         # same-label column window per m-tile
TINV = 10.0       # 1 / temperature
F32 = mybir.dt.float32
BF16 = mybir.dt.bfloat16

_CACHE = {}


def _build_nc():
    nc = bacc.Bacc()
    et = nc.dram_tensor("et", [D, B], BF16, kind="ExternalInput")
    elt = nc.dram_tensor("elt", [D, BL], BF16, kind="ExternalInput")
    etwin = nc.dram_tensor("etwin", [NMT, 128, NK * WIN], BF16, kind="ExternalInput")
    colwin = nc.dram_tensor("colwin", [NMT, 128, WIN], BF16, kind="ExternalInput")
    meta = nc.dram_tensor("meta", [NMT, 128, 4], F32, kind="ExternalInput")
    out = nc.dram_tensor("out", [NMT, 128, 1], F32, kind="ExternalOutput")

    AF = mybir.ActivationFunctionType
    OP = mybir.AluOpType

    with tile.TileContext(nc) as tc:
        with (
            tc.tile_pool(name="const", bufs=1) as cpool,
            tc.tile_pool(name="psum", bufs=3, space=bass.MemorySpace.PSUM) as ppool,
            tc.tile_pool(name="psumw", bufs=2, space=bass.MemorySpace.PSUM) as pwpool,
            tc.tile_pool(name="chunks", bufs=3) as chpool,
            tc.tile_pool(name="winp", bufs=2) as wpool,
            tc.tile_pool(name="small", bufs=2) as smpool,
        ):
            ets = [cpool.tile([128, B], BF16, tag=f"ets{k}", name=f"ets{k}")
                   for k in range(NK)]
            eltt = [cpool.tile([128, BL], BF16, tag=f"elt{k}", name=f"elt{k}")
                    for k in range(NK)]
            etw_sb = [cpool.tile([128, NK * WIN], BF16, tag=f"etw{m}",
                                 name=f"etw{m}") for m in range(NMT)]
            colw_sb = [cpool.tile([128, WIN], BF16, tag=f"colw{m}", name=f"colw{m}")
                       for m in range(NMT)]
            meta_sb = [cpool.tile([128, 4], F32, tag=f"meta{m}", name=f"meta{m}")
                       for m in range(NMT)]

            # Loads on the two HWDGE queues (SP + Act); gpsimd SWDGE issue
            # is ~1us/DMA and would gate the pipeline. First-chunk data
            # (lhsT k-tiles + first column halves) interleave up front.
            HB = B // 2
            for k in range(NK):
                nc.sync.dma_start(eltt[k][:], elt[k * 128:(k + 1) * 128, :])
                nc.sync.dma_start(
                    ets[k][:, 0:HB], et[k * 128:(k + 1) * 128, 0:HB])
            for k in range(NK):
                nc.scalar.dma_start(
                    ets[k][:, HB:B], et[k * 128:(k + 1) * 128, HB:B])
            for m in range(NMT):
                nc.scalar.dma_start(meta_sb[m][:], meta[m])
                nc.scalar.dma_start(colw_sb[m][:], colwin[m])
                nc.scalar.dma_start(etw_sb[m][:], etwin[m])

            sexps, mews, denoms, invs = [], [], [], []
            # ---- Phase A (Exp table set): dense accums + window pipeline --
            for mt in range(NMT):
                rowlab = meta_sb[mt][:, 0:1]
                aparts = smpool.tile([128, NCH], F32, tag="aparts")

                # dense: sum_j exp(s_ij) via fused accumulate, output dead
                for c in range(NCH):
                    psum = ppool.tile([128, CH], F32, tag="psum")
                    for k in range(NK):
                        lhsT = eltt[k][:, mt * 128:(mt + 1) * 128]
                        for h in range(CH // 512):
                            col0 = c * CH + h * 512
                            nc.tensor.matmul(
                                psum[:, h * 512:(h + 1) * 512],
                                lhsT,
                                ets[k][:, col0:col0 + 512],
                                start=(k == 0),
                                stop=(k == NK - 1),
                            )
                    dead = chpool.tile([128, CH], BF16, tag="dead")
                    nc.scalar.activation(
                        dead[:], psum[:], AF.Exp,
                        scale=TINV, accum_out=aparts[:, c:c + 1],
                    )

                # window: recompute the <=512 same-label columns
                psw = pwpool.tile([128, WIN], F32, tag="psw")
                for k in range(NK):
                    nc.tensor.matmul(
                        psw[:],
                        eltt[k][:, mt * 128:(mt + 1) * 128],
                        etw_sb[mt][:, k * WIN:(k + 1) * WIN],
                        start=(k == 0),
                        stop=(k == NK - 1),
                    )
                expw = wpool.tile([128, WIN], F32, tag="expw")
                last_a_act = nc.scalar.activation(
                    expw[:], psw[:], AF.Exp, scale=TINV)
                maskw = wpool.tile([128, WIN], BF16, tag="maskw")
                nc.vector.tensor_scalar(
                    maskw[:], colw_sb[mt][:], rowlab, None, OP.is_equal)
                mew = wpool.tile([128, WIN], F32, tag=f"mew{mt}", name=f"mew{mt}",
                                 bufs=1)
                nc.vector.tensor_tensor(mew[:], expw[:], maskw[:], OP.mult)
                ssame = smpool.tile([128, 1], F32, tag="ssame")
                nc.vector.tensor_reduce(
                    ssame[:], mew[:], mybir.AxisListType.X, OP.add)

                sall = smpool.tile([128, 1], F32, tag="sall")
                nc.vector.tensor_reduce(
                    sall[:], aparts[:], mybir.AxisListType.X, OP.add)
                denom = smpool.tile([128, 1], F32, tag=f"denom{mt}",
                                    name=f"denom{mt}")
                nc.vector.tensor_sub(denom[:], sall[:], ssame[:])
                inv = smpool.tile([128, 1], F32, tag=f"inv{mt}", name=f"inv{mt}")
                nc.vector.reciprocal(inv[:], denom[:])
                mews.append(mew); denoms.append(denom); invs.append(inv)

            # ---- Phase B (Ln table set): all log work batched ----
            lnouts = wpool.tile([128, WIN], BF16, tag="lnout", bufs=1)
            for mt in range(NMT):
                cnt = meta_sb[mt][:, 1:2]
                msum = meta_sb[mt][:, 2:3]
                eii = meta_sb[mt][:, 3:4]
                denom, inv, mew = denoms[mt], invs[mt], mews[mt]

                lnden = smpool.tile([128, 1], F32, tag=f"lnden{mt}",
                                    name=f"lnden{mt}")
                i_ld = nc.scalar.activation(lnden[:], denom[:], AF.Ln)
                tile.add_dep_helper(i_ld.ins, last_a_act.ins, sync=False,
                                    reason="keep Ln set after all Exp work")
                lndiag = smpool.tile([128, 1], F32, tag=f"lndiag{mt}",
                                     name=f"lndiag{mt}")
                i_lg = nc.scalar.activation(lndiag[:], eii, AF.Ln, bias=denom[:])
                tile.add_dep_helper(i_lg.ins, last_a_act.ins, sync=False,
                                    reason="keep Ln set after all Exp work")
                slog = smpool.tile([128, 1], F32, tag=f"slog{mt}",
                                   name=f"slog{mt}")
                i_sl = nc.scalar.activation(
                    lnouts[:], mew[:], AF.Ln,
                    scale=inv[:], bias=1.0, accum_out=slog[:],
                )
                tile.add_dep_helper(i_sl.ins, last_a_act.ins, sync=False,
                                    reason="keep Ln set after all Exp work")

                # rowtot = ((cnt*lnden + slog) - lndiag) + (sii - rds)
                ta = smpool.tile([128, 1], F32, tag=f"ta{mt}", name=f"ta{mt}")
                nc.vector.tensor_scalar(
                    ta[:], lnden[:], cnt, slog[:, 0:1], OP.mult, OP.add)
                rowtot = smpool.tile([128, 1], F32, tag=f"rowtot{mt}",
                                     name=f"rowtot{mt}")
                nc.vector.tensor_scalar(
                    rowtot[:], ta[:], lndiag[:, 0:1], msum, OP.subtract, OP.add)
                nc.sync.dma_start(out[mt], rowtot[:])
    nc.compile()
    return nc


def _make_in_maps(embeddings, labels):
    """Host-side prep: label-sort, transposes, windows, per-row scalars,
    per-core input dicts. Returns (in_maps, num_pos)."""
    emb0 = np.ascontiguousarray(np.asarray(embeddings, dtype=np.float32))
    lab0 = np.asarray(labels).astype(np.int64)
    assert emb0.shape == (B, D) and lab0.shape == (B,)

    perm = np.argsort(lab0, kind="stable")
    emb = emb0[perm]
    lab = lab0[perm]

    ET = np.ascontiguousarray(emb.T)                      # [D, B], sorted cols
    ET16 = ET.astype(ml_dtypes.bfloat16)
    labf = lab.astype(np.float32)
    lab16 = labf.astype(ml_dtypes.bfloat16)

    ncls = int(lab.max()) + 1
    counts = np.bincount(lab, minlength=ncls)
    cum = np.concatenate([[0], np.cumsum(counts)])
    cnt = counts[lab].astype(np.float64)                  # same-label count incl. self
    num_pos = float(cnt.sum() - B)

    emb64 = emb.astype(np.float64)
    G = np.zeros((ncls, D), np.float64)
    np.add.at(G, lab, emb64)
    rds = (emb64 * G[lab]).sum(1) * TINV                  # sum_{j same} sims_ij / T
    sii = (emb64 * emb64).sum(1) * TINV                   # sims_ii / T

    meta_all = np.stack(
        [labf.astype(np.float64), cnt, sii - rds, np.exp(sii)], axis=-1
    ).astype(np.float32)                                  # [B, 4]

    in_maps = []
    for c in range(NCORES):
        sl = slice(c * BL, (c + 1) * BL)
        etwin = np.zeros((NMT, D, WIN), ml_dtypes.bfloat16)
        colwin = np.zeros((NMT, 128, WIN), ml_dtypes.bfloat16)
        colwin[:, :, :] = ml_dtypes.bfloat16(-1.0)        # never matches a label
        for m in range(NMT):
            r0 = c * BL + m * 128
            c0 = int(cum[lab[r0]])
            c1 = int(cum[lab[r0 + 127] + 1])
            w = c1 - c0
            assert w <= WIN, f"window {w} exceeds {WIN}; rebuild with larger WIN"
            etwin[m, :, :w] = ET16[:, c0:c1]
            colwin[m, :, :w] = lab16[c0:c1][None, :]
        etwin_packed = np.ascontiguousarray(
            etwin.reshape(NMT, NK, 128, WIN).transpose(0, 2, 1, 3)
            .reshape(NMT, 128, NK * WIN))
        in_maps.append({
            "et": ET16,
            "elt": np.ascontiguousarray(ET16[:, sl]),
            "etwin": etwin_packed,
            "colwin": colwin,
            "meta": np.ascontiguousarray(meta_all[sl].reshape(NMT, 128, 4)),
        })
    return in_maps, num_pos


def kernel(embeddings, labels):
    in_maps, num_pos = _make_in_maps(embeddings, labels)
    if "nc" not in _CACHE:
        _CACHE["nc"] = _build_nc()
    nc = _CACHE["nc"]
    res = run_bass_kernel_spmd(nc, in_maps, list(range(NCORES)))
    total = sum(float(r["out"].sum()) for r in res.results)
    return np.asarray(total / max(num_pos, 1.0), dtype=np.float32)


# revision 20
# speedup vs baseline: 1.6089x; 1.1732x over previous
"""Supervised contrastive loss on 8 Trainium2 NeuronCores.

Strategy (data-parallel over embedding rows, per the sharding hint), with a
label-sorted layout so the masked work collapses to narrow windows:

  - The host sorts rows AND columns by label (the loss is permutation
    invariant). Each core owns 512 sorted rows; each 128-row m-tile's
    same-label partners then live in ONE contiguous column window of at
    most 512 columns (multinomial counts make wider windows essentially
    impossible; asserted on the host).
  - Dense path: bf16 PE matmuls compute the [128, B] similarity slab in
    PSUM chunks; one ACT Exp pass per chunk (scale=1/T) with the fused
    per-row accumulate yields sum_j exp(s_ij). The Exp output is a dead
    store - only the accumulator is consumed.
  - Window path: 4 small matmuls recompute the window's sims (bit-identical
    inputs), then exp / is_equal mask / masked-multiply / row-reduce /
    log1p run on [128, 512] tiles only (~1/8 of the columns):
      sum_same = sum_win mask * exp;  denom = sum_all - sum_same
      slog = sum_win ln(1 + me * (1/denom))   [diagonal included]
  - Per-row loss: cnt_i*ln(denom_i) + slog_i - ln(denom_i + e^{s_ii})
                  - sum_{j same} s_ij + s_ii
    where cnt, s_ii, e^{s_ii} and sum_{j same} s_ij (via class-sum matrix
    G) are exact O(B*D) host precomputes.
  - Each core writes its 512 per-row contributions; the host sums 4096
    values and divides by num_pos (exact, from label counts).
"""

import ml_dtypes
import numpy as np

import concourse.bass as bass
import concourse.bacc as bacc
import concourse.mybir as mybir
import concourse.tile as tile
from concourse.bass_utils import run_bass_kernel_spmd

B = 4096          # total rows
D = 512           # embedding dim
NCORES = 8
BL = B // NCORES  # rows per core
NK = D // 128     # contraction k-tiles
NMT = BL // 128   # output m-tiles per core
CH = 1024         # dense column chunk (2 PSUM banks)
NCH = B // CH     # dense chunks per m-tile row
WIN = 512         # same-label column window per m-tile
TINV = 10.0       # 1 / temperature
F32 = mybir.dt.float32
BF16 = mybir.dt.bfloat16
F8 = mybir.dt.float8e4
NP_F8 = mybir.dt.np(F8)
SCALE = 16.0      # fp8 pre-scale; folded out via the Exp activation scale

_CACHE = {}


def _build_nc():
    nc = bacc.Bacc()
    et = nc.dram_tensor("et", [D, B], F8, kind="ExternalInput")
    elt = nc.dram_tensor("elt", [D, BL], F8, kind="ExternalInput")
    etwin = nc.dram_tensor("etwin", [NMT, 128, NK * WIN], F8, kind="ExternalInput")
    colwin = nc.dram_tensor("colwin", [NMT, 128, WIN], BF16, kind="ExternalInput")
    meta = nc.dram_tensor("meta", [NMT, 128, 4], F32, kind="ExternalInput")
    out = nc.dram_tensor("out", [128, NMT], F32, kind="ExternalOutput")

    AF = mybir.ActivationFunctionType
    OP = mybir.AluOpType

    with tile.TileContext(nc) as tc:
        with (
            tc.tile_pool(name="const", bufs=1) as cpool,
            tc.tile_pool(name="psum", bufs=3, space=bass.MemorySpace.PSUM) as ppool,
            tc.tile_pool(name="psumw", bufs=2, space=bass.MemorySpace.PSUM) as pwpool,
            tc.tile_pool(name="chunks", bufs=3) as chpool,
            tc.tile_pool(name="winp", bufs=2) as wpool,
            tc.tile_pool(name="small", bufs=2) as smpool,
        ):
            ets = [cpool.tile([128, B], F8, tag=f"ets{k}", name=f"ets{k}")
                   for k in range(NK)]
            eltt = [cpool.tile([128, BL], F8, tag=f"elt{k}", name=f"elt{k}")
                    for k in range(NK)]
            etw_sb = [cpool.tile([128, NK * WIN], F8, tag=f"etw{m}",
                                 name=f"etw{m}") for m in range(NMT)]
            colw_sb = [cpool.tile([128, WIN], BF16, tag=f"colw{m}", name=f"colw{m}")
                       for m in range(NMT)]
            meta_sb = [cpool.tile([128, 4], F32, tag=f"meta{m}", name=f"meta{m}")
                       for m in range(NMT)]

            # Loads on the two HWDGE queues (SP + Act); gpsimd SWDGE issue
            # is ~1us/DMA and would gate the pipeline. First-chunk data
            # (lhsT k-tiles + first column halves) interleave up front.
            HB = B // 2
            for k in range(NK):
                nc.sync.dma_start(eltt[k][:], elt[k * 128:(k + 1) * 128, :])
                nc.sync.dma_start(
                    ets[k][:, 0:HB], et[k * 128:(k + 1) * 128, 0:HB])
            for k in range(NK):
                nc.scalar.dma_start(
                    ets[k][:, HB:B], et[k * 128:(k + 1) * 128, HB:B])
            for m in range(NMT):
                nc.scalar.dma_start(meta_sb[m][:], meta[m])
                nc.scalar.dma_start(colw_sb[m][:], colwin[m])
                nc.scalar.dma_start(etw_sb[m][:], etwin[m])

            sexps, mews, denoms, invs = [], [], [], []
            # ---- Phase A (Exp table set): dense accums + window pipeline --
            for mt in range(NMT):
                rowlab = meta_sb[mt][:, 0:1]
                aparts = smpool.tile([128, NCH], F32, tag="aparts")

                # dense: sum_j exp(s_ij) via fused accumulate, output dead
                for c in range(NCH):
                    psum = ppool.tile([128, CH], F32, tag="psum")
                    for k in range(NK):
                        lhsT = eltt[k][:, mt * 128:(mt + 1) * 128]
                        for h in range(CH // 512):
                            col0 = c * CH + h * 512
                            nc.tensor.matmul(
                                psum[:, h * 512:(h + 1) * 512],
                                lhsT,
                                ets[k][:, col0:col0 + 512],
                                start=(k == 0),
                                stop=(k == NK - 1),
                            )
                    dead = chpool.tile([128, CH], BF16, tag="dead")
                    nc.scalar.activation(
                        dead[:], psum[:], AF.Exp,
                        scale=TINV / (SCALE * SCALE),
                        accum_out=aparts[:, c:c + 1],
                    )

                # window: recompute the <=512 same-label columns
                psw = pwpool.tile([128, WIN], F32, tag="psw")
                for k in range(NK):
                    nc.tensor.matmul(
                        psw[:],
                        eltt[k][:, mt * 128:(mt + 1) * 128],
                        etw_sb[mt][:, k * WIN:(k + 1) * WIN],
                        start=(k == 0),
                        stop=(k == NK - 1),
                    )
                expw = wpool.tile([128, WIN], F32, tag="expw")
                last_a_act = nc.scalar.activation(
                    expw[:], psw[:], AF.Exp, scale=TINV / (SCALE * SCALE))
                maskw = wpool.tile([128, WIN], BF16, tag="maskw")
                nc.vector.tensor_scalar(
                    maskw[:], colw_sb[mt][:], rowlab, None, OP.is_equal)
                mew = wpool.tile([128, WIN], F32, tag=f"mew{mt}", name=f"mew{mt}",
                                 bufs=1)
                nc.vector.tensor_tensor(mew[:], expw[:], maskw[:], OP.mult)
                ssame = smpool.tile([128, 1], F32, tag="ssame")
                nc.vector.tensor_reduce(
                    ssame[:], mew[:], mybir.AxisListType.X, OP.add)

                sall = smpool.tile([128, 1], F32, tag="sall")
                nc.vector.tensor_reduce(
                    sall[:], aparts[:], mybir.AxisListType.X, OP.add)
                denom = smpool.tile([128, 1], F32, tag=f"denom{mt}",
                                    name=f"denom{mt}")
                nc.vector.tensor_sub(denom[:], sall[:], ssame[:])
                inv = smpool.tile([128, 1], F32, tag=f"inv{mt}", name=f"inv{mt}")
                nc.vector.reciprocal(inv[:], denom[:])
                mews.append(mew); denoms.append(denom); invs.append(inv)

            # ---- Phase B (Ln table set): all log work batched ----
            lnouts = wpool.tile([128, WIN], BF16, tag="lnout", bufs=1)
            rowtots = wpool.tile([128, NMT], F32, tag="rowtots", bufs=1)
            for mt in range(NMT):
                cnt = meta_sb[mt][:, 1:2]
                msum = meta_sb[mt][:, 2:3]
                eii = meta_sb[mt][:, 3:4]
                denom, inv, mew = denoms[mt], invs[mt], mews[mt]

                lnden = smpool.tile([128, 1], F32, tag=f"lnden{mt}",
                                    name=f"lnden{mt}")
                i_ld = nc.scalar.activation(lnden[:], denom[:], AF.Ln)
                tile.add_dep_helper(i_ld.ins, last_a_act.ins, sync=False,
                                    reason="keep Ln set after all Exp work")
                lndiag = smpool.tile([128, 1], F32, tag=f"lndiag{mt}",
                                     name=f"lndiag{mt}")
                i_lg = nc.scalar.activation(lndiag[:], eii, AF.Ln, bias=denom[:])
                tile.add_dep_helper(i_lg.ins, last_a_act.ins, sync=False,
                                    reason="keep Ln set after all Exp work")
                slog = smpool.tile([128, 1], F32, tag=f"slog{mt}",
                                   name=f"slog{mt}")
                i_sl = nc.scalar.activation(
                    lnouts[:], mew[:], AF.Ln,
                    scale=inv[:], bias=1.0, accum_out=slog[:],
                )
                tile.add_dep_helper(i_sl.ins, last_a_act.ins, sync=False,
                                    reason="keep Ln set after all Exp work")

                # rowtot = ((cnt*lnden + slog) - lndiag) + (sii - rds)
                ta = smpool.tile([128, 1], F32, tag=f"ta{mt}", name=f"ta{mt}")
                nc.vector.tensor_scalar(
                    ta[:], lnden[:], cnt, slog[:, 0:1], OP.mult, OP.add)
                nc.vector.tensor_scalar(
                    rowtots[:, mt:mt + 1], ta[:], lndiag[:, 0:1], msum,
                    OP.subtract, OP.add)
            nc.sync.dma_start(out[:], rowtots[:])
    nc.compile()
    return nc


def _make_in_maps(embeddings, labels):
    """Host-side prep: label-sort, transposes, windows, per-row scalars,
    per-core input dicts. Returns (in_maps, num_pos)."""
    emb0 = np.ascontiguousarray(np.asarray(embeddings, dtype=np.float32))
    lab0 = np.asarray(labels).astype(np.int64)
    assert emb0.shape == (B, D) and lab0.shape == (B,)

    perm = np.argsort(lab0, kind="stable")
    emb = emb0[perm]
    lab = lab0[perm]

    ET = np.ascontiguousarray(emb.T)                      # [D, B], sorted cols
    ET8 = (ET * SCALE).astype(NP_F8)
    labf = lab.astype(np.float32)
    lab16 = labf.astype(ml_dtypes.bfloat16)

    ncls = int(lab.max()) + 1
    counts = np.bincount(lab, minlength=ncls)
    cum = np.concatenate([[0], np.cumsum(counts)])
    cnt = counts[lab].astype(np.float64)                  # same-label count incl. self
    num_pos = float(cnt.sum() - B)

    emb64 = emb.astype(np.float64)
    G = np.zeros((ncls, D), np.float64)
    np.add.at(G, lab, emb64)
    rds = (emb64 * G[lab]).sum(1) * TINV                  # sum_{j same} sims_ij / T
    sii = (emb64 * emb64).sum(1) * TINV                   # sims_ii / T

    meta_all = np.stack(
        [labf.astype(np.float64), cnt, sii - rds, np.exp(sii)], axis=-1
    ).astype(np.float32)                                  # [B, 4]

    in_maps = []
    for c in range(NCORES):
        sl = slice(c * BL, (c + 1) * BL)
        etwin = np.zeros((NMT, D, WIN), NP_F8)
        colwin = np.zeros((NMT, 128, WIN), ml_dtypes.bfloat16)
        colwin[:, :, :] = ml_dtypes.bfloat16(-1.0)        # never matches a label
        for m in range(NMT):
            r0 = c * BL + m * 128
            c0 = int(cum[lab[r0]])
            c1 = int(cum[lab[r0 + 127] + 1])
            w = c1 - c0
            assert w <= WIN, f"window {w} exceeds {WIN}; rebuild with larger WIN"
            etwin[m, :, :w] = ET8[:, c0:c1]
            colwin[m, :, :w] = lab16[c0:c1][None, :]
        etwin_packed = np.ascontiguousarray(
            etwin.reshape(NMT, NK, 128, WIN).transpose(0, 2, 1, 3)
            .reshape(NMT, 128, NK * WIN))
        in_maps.append({
            "et": ET8,
            "elt": np.ascontiguousarray(ET8[:, sl]),
            "etwin": etwin_packed,
            "colwin": colwin,
            "meta": np.ascontiguousarray(meta_all[sl].reshape(NMT, 128, 4)),
        })
    return in_maps, num_pos


def kernel(embeddings, labels):
    in_maps, num_pos = _make_in_maps(embeddings, labels)
    if "nc" not in _CACHE:
        _CACHE["nc"] = _build_nc()
    nc = _CACHE["nc"]
    res = run_bass_kernel_spmd(nc, in_maps, list(range(NCORES)))
    total = sum(float(r["out"].sum()) for r in res.results)
    return np.asarray(total / max(num_pos, 1.0), dtype=np.float32)
